# revision 1
# baseline (speedup 1.0000x reference)
"""GraphTransformer (2x PyG TransformerConv + linear) on 8 trn2 NeuronCores.

Strategy: edges sorted by destination, packed into fixed-size blocks
(512 edge slots / 64 dst slots, segments never split). Contiguous dst
ranges are sharded across the 8 cores (edge-balanced). Three SPMD
launches:
  P : per-core slice projections  kv1 = x@[Wk1|Wv1],  qs1 = x@[Wq1|Ws1]
  L1: per-edge gather of kv1 rows (indirect DMA), attention via PE
      matmuls against per-block one-hot segment matrices M, softmax
      without max-subtraction (scores are O(1)), segment sums via
      M^T @ [weighted_v || exp], dense normalize+skip+relu pass, then
      the layer-2 projection kvqs2 = h1@[Wk2|Wv2|Wq2|Ws2]
  L2: same loop on kv2 (single head), final linear to [N, 2]
Host does only index bookkeeping, concatenation and transposes.
"""
import sys

sys.path.insert(0, "/opt/trn_rl_repo")
import numpy as np
import concourse.bass as bass
import concourse.bacc as bacc
import concourse.tile as tile
from concourse import mybir
from concourse.bass_utils import run_bass_kernel_spmd
from concourse.masks import make_identity

F32 = mybir.dt.float32
I32 = mybir.dt.int32
NCORES = 8
NNODE = 50000
EPB, DPB, TSUB = 512, 64, 4          # edges/block, dst slots/block, 128-edge subtiles
NOWN = 6400                          # padded own-node slots per core (50*128)
NTIL = NOWN // 128

_built = {}


def _bc(ap, p):
    """Broadcast a [1, n] DRAM AP across p partitions."""
    return bass.AP(tensor=ap.tensor, offset=ap.offset, ap=[[0, p]] + list(ap.ap[1:]))


def _build_P():
    nc = bacc.Bacc()
    xT = nc.declare_dram_parameter("xT", [64, NOWN], F32, isOutput=False)
    W1 = nc.declare_dram_parameter("W1", [64, 1024], F32, isOutput=False)
    b1 = nc.declare_dram_parameter("b1", [1, 1024], F32, isOutput=False)
    kv = nc.declare_dram_parameter("kv", [NOWN, 512], F32, isOutput=True)
    qs = nc.declare_dram_parameter("qs", [NOWN, 512], F32, isOutput=True)
    with tile.TileContext(nc) as tc:
        with tc.tile_pool(name="one", bufs=1) as one, \
             tc.tile_pool(name="sb", bufs=3) as sb, \
             tc.tile_pool(name="ps", bufs=2, space="PSUM") as ps:
            W1t = one.tile([64, 1024], F32)
            nc.sync.dma_start(out=W1t[:], in_=W1[:])
            b1t = one.tile([128, 1024], F32)
            nc.sync.dma_start(out=b1t[:], in_=_bc(b1[:], 128))
            for i in range(NTIL):
                r = slice(i * 128, (i + 1) * 128)
                xt = sb.tile([64, 128], F32, tag="xt")
                nc.sync.dma_start(out=xt[:], in_=xT[:, r])
                o_kv = sb.tile([128, 512], F32, tag="okv")
                o_qs = sb.tile([128, 512], F32, tag="oqs")
                for j, od in ((0, o_kv), (1, o_qs)):
                    pp = ps.tile([128, 512], F32, tag=f"p{j}")
                    nc.tensor.matmul(out=pp[:], lhsT=xt[:],
                                     rhs=W1t[:, j * 512:(j + 1) * 512],
                                     start=True, stop=True)
                    nc.vector.tensor_add(od[:], pp[:], b1t[:, j * 512:(j + 1) * 512])
                nc.sync.dma_start(out=kv[r, :], in_=o_kv[:])
                nc.sync.dma_start(out=qs[r, :], in_=o_qs[:])
    nc.finalize()
    return nc


def _build_conv(B, DKV, H, OUTW):
    """Gather/attention launch. DKV: gathered row width (k|v), H heads of 64.
    OUTW: trailing dense-output width (256 for L1's kvqs2, 2 for L2's final)."""
    DH = DKV // 2                     # features per head-group (k or v part)
    SW = DKV // 2 + H                 # stage row: msg(DH) + per-head sums(H)
    nc = bacc.Bacc()
    kvf = nc.declare_dram_parameter("kvf", [NNODE, DKV], F32, isOutput=False)
    qtab = nc.declare_dram_parameter("qtab", [NOWN + 1, DH], F32, isOutput=False)
    sktab = nc.declare_dram_parameter("sktab", [NOWN, DH], F32, isOutput=False)
    srcb = nc.declare_dram_parameter("srcb", [B, 128, TSUB], I32, isOutput=False)
    Mb = nc.declare_dram_parameter("Mb", [B, 128, TSUB * DPB], F32, isOutput=False)
    qrow = nc.declare_dram_parameter("qrow", [B, DPB, 1], I32, isOutput=False)
    strow = nc.declare_dram_parameter("strow", [NOWN, 1], I32, isOutput=False)
    WO = nc.declare_dram_parameter("WO", [DH, OUTW], F32, isOutput=False)
    bO = nc.declare_dram_parameter("bO", [1, OUTW], F32, isOutput=False)
    outt = nc.declare_dram_parameter("outt", [NOWN, OUTW], F32, isOutput=True)
    stage = nc.dram_tensor("stage", [B * DPB, SW], F32)

    with tile.TileContext(nc) as tc:
        with tc.tile_pool(name="one", bufs=1) as one:
            ident = one.tile([128, 128], F32)
            make_identity(nc, ident[:])
            nwo = (DH + 127) // 128
            WOt = [one.tile([min(128, DH - 128 * k), OUTW], F32, tag=f"wo{k}",
                            name=f"wo{k}")
                   for k in range(nwo)]
            for k in range(nwo):
                nc.sync.dma_start(out=WOt[k][:], in_=WO[128 * k: 128 * k + WOt[k].shape[0], :])
            bOt = one.tile([128, OUTW], F32)
            nc.sync.dma_start(out=bOt[:], in_=_bc(bO[:], 128))

            # ---- block loop: gather + attention partial sums ----
            with tc.tile_pool(name="sb", bufs=3) as sb, \
                 tc.tile_pool(name="ps", bufs=2, space="PSUM") as ps:
                for b in range(B):
                    src_t = sb.tile([128, TSUB], I32, tag="src")
                    nc.sync.dma_start(out=src_t[:], in_=srcb[b])
                    qr_t = sb.tile([DPB, 1], I32, tag="qr")
                    nc.sync.dma_start(out=qr_t[:], in_=qrow[b])
                    M_t = sb.tile([128, TSUB * DPB], F32, tag="M")
                    nc.sync.dma_start(out=M_t[:], in_=Mb[b])
                    qrows = sb.tile([DPB, DH], F32, tag="qrows")
                    nc.gpsimd.indirect_dma_start(
                        out=qrows[:], out_offset=None, in_=qtab[:],
                        in_offset=bass.IndirectOffsetOnAxis(ap=qr_t[:, :1], axis=0))
                    vwe = sb.tile([128, TSUB, DH + H], F32, tag="vwe")
                    agg = ps.tile([DPB, SW], F32, tag="agg")
                    for t in range(TSUB):
                        kvt = sb.tile([128, DKV], F32, tag=f"kv{t}")
                        nc.gpsimd.indirect_dma_start(
                            out=kvt[:], out_offset=None, in_=kvf[:],
                            in_offset=bass.IndirectOffsetOnAxis(
                                ap=src_t[:, t:t + 1], axis=0))
                        Mcol = M_t[:, t * DPB:(t + 1) * DPB]
                        mtp = ps.tile([DPB, 128], F32, tag="mt")
                        nc.tensor.transpose(out=mtp[:], in_=Mcol, identity=ident[:])
                        mts = sb.tile([DPB, 128], F32, tag="mts")
                        nc.vector.tensor_copy(mts[:], mtp[:])
                        qep = ps.tile([128, DH], F32, tag="qe")
                        nc.tensor.matmul(out=qep[:], lhsT=mts[:], rhs=qrows[:],
                                         start=True, stop=True)
                        prod = sb.tile([128, DH], F32, tag="prod")
                        nc.vector.tensor_mul(prod[:], qep[:], kvt[:, 0:DH])
                        alpha = sb.tile([128, H], F32, tag="alpha")
                        nc.vector.reduce_sum(
                            out=alpha[:],
                            in_=prod[:].rearrange("p (h d) -> p h d", h=H),
                            axis=mybir.AxisListType.X)
                        expv = vwe[:, t, DH:DH + H]
                        nc.scalar.activation(expv, alpha[:],
                                             mybir.ActivationFunctionType.Exp,
                                             scale=0.125)
                        nc.vector.tensor_mul(
                            vwe[:, t, 0:DH].rearrange("p (h d) -> p h d", h=H),
                            kvt[:, DH:DKV].rearrange("p (h d) -> p h d", h=H),
                            expv.unsqueeze(2).to_broadcast([128, H, 64]))
                        nc.tensor.matmul(out=agg[:], lhsT=Mcol, rhs=vwe[:, t, :],
                                         start=(t == 0), stop=(t == TSUB - 1))
                    aggs = sb.tile([DPB, SW], F32, tag="aggs")
                    nc.vector.tensor_copy(aggs[:], agg[:])
                    nc.sync.dma_start(out=stage[b * DPB:(b + 1) * DPB, :], in_=aggs[:])

            # ---- dense pass: normalize + skip + relu + output matmul ----
            with tc.tile_pool(name="sb2", bufs=3) as sb, \
                 tc.tile_pool(name="ps2", bufs=2, space="PSUM") as ps:
                for i in range(NTIL):
                    r = slice(i * 128, (i + 1) * 128)
                    st_t = sb.tile([128, 1], I32, tag="st")
                    nc.sync.dma_start(out=st_t[:], in_=strow[r])
                    pre = sb.tile([128, SW], F32, tag="pre")
                    nc.gpsimd.indirect_dma_start(
                        out=pre[:], out_offset=None, in_=stage[:],
                        in_offset=bass.IndirectOffsetOnAxis(ap=st_t[:, :1], axis=0))
                    sc = sb.tile([128, H], F32, tag="sc")
                    nc.vector.tensor_scalar_max(sc[:], pre[:, DH:SW], 1e-30)
                    rs = sb.tile([128, H], F32, tag="rs")
                    nc.vector.reciprocal(rs[:], sc[:])
                    sk = sb.tile([128, DH], F32, tag="sk")
                    nc.sync.dma_start(out=sk[:], in_=sktab[r, :])
                    h = sb.tile([128, DH], F32, tag="h")
                    nc.vector.tensor_mul(
                        h[:].rearrange("p (g d) -> p g d", g=H),
                        pre[:, 0:DH].rearrange("p (g d) -> p g d", g=H),
                        rs[:].unsqueeze(2).to_broadcast([128, H, 64]))
                    nc.vector.tensor_add(h[:], h[:], sk[:])
                    nc.scalar.activation(h[:], h[:],
                                         mybir.ActivationFunctionType.Relu)
                    op = ps.tile([128, OUTW], F32, tag="op")
                    for k in range(nwo):
                        kw = WOt[k].shape[0]
                        tp = ps.tile([kw, 128], F32, tag="tp")
                        nc.tensor.transpose(out=tp[:], in_=h[:, 128 * k:128 * k + kw],
                                            identity=ident[:])
                        ts_ = sb.tile([kw, 128], F32, tag="ts")
                        nc.vector.tensor_copy(ts_[:], tp[:])
                        nc.tensor.matmul(out=op[:], lhsT=ts_[:], rhs=WOt[k][:],
                                         start=(k == 0), stop=(k == nwo - 1))
                    oo = sb.tile([128, OUTW], F32, tag="oo")
                    nc.vector.tensor_add(oo[:], op[:], bOt[:])
                    nc.sync.dma_start(out=outt[r, :], in_=oo[:])
    nc.finalize()
    return nc


def _prep(edge_index):
    """Sort/pack the graph. Returns per-core block metadata."""
    src = np.ascontiguousarray(edge_index[0]).astype(np.int64)
    dst = np.ascontiguousarray(edge_index[1]).astype(np.int64)
    E = src.shape[0]
    order = np.argsort(dst, kind="stable")
    s_sorted = src[order].astype(np.int32)
    d_sorted = dst[order]
    deg = np.bincount(d_sorted, minlength=NNODE)
    cume = np.concatenate([[0], np.cumsum(deg)])          # edge start per node
    # core boundaries: balanced edge counts at node granularity
    targets = [round(E * c / NCORES) for c in range(1, NCORES)]
    nb = [0] + [int(np.searchsorted(cume, t)) for t in targets] + [NNODE]
    cores = []
    for c in range(NCORES):
        n0, n1 = nb[c], nb[c + 1]
        assert n1 - n0 <= NOWN, (c, n1 - n0)
        blocks = []   # list of (list of (node, edge_lo, edge_hi))
        cur, ecnt = [], 0
        for n in range(n0, n1):
            g = int(deg[n])
            assert g <= EPB
            if len(cur) >= DPB or ecnt + g > EPB:
                blocks.append(cur)
                cur, ecnt = [], 0
            cur.append(n)
            ecnt += g
        if cur:
            blocks.append(cur)
        cores.append((n0, n1, blocks))
    B = max(len(cb) for _, _, cb in cores)
    per_core = []
    for c in range(NCORES):
        n0, n1, blocks = cores[c]
        srcb = np.zeros((B, EPB), np.int32)
        Mb = np.zeros((B, 128, TSUB * DPB), np.float32)
        qrow = np.full((B, DPB, 1), NOWN, np.int32)
        strow = np.zeros((NOWN, 1), np.int32)
        for b, nodes in enumerate(blocks):
            e = 0
            for slot, n in enumerate(nodes):
                qrow[b, slot, 0] = n - n0
                strow[n - n0, 0] = b * DPB + slot
                lo, hi = cume[n], cume[n + 1]
                g = hi - lo
                srcb[b, e:e + g] = s_sorted[lo:hi]
                for k in range(g):
                    ee = e + k
                    Mb[b, ee % 128, (ee // 128) * DPB + slot] = 1.0
                e += g
        # edge slot e -> subtile e//128, partition e%128
        srcb = srcb.reshape(B, TSUB, 128).transpose(0, 2, 1).copy()
        per_core.append(dict(n0=n0, n1=n1, srcb=srcb, Mb=Mb, qrow=qrow,
                             strow=strow))
    return B, per_core


def kernel(x, edge_index, Wq1, bq1, Wk1, bk1, Wv1, bv1, Ws1, bs1,
           Wq2, bq2, Wk2, bk2, Wv2, bv2, Ws2, bs2, Wl, bl):
    x = np.asarray(x, np.float32)
    B, per_core = _prep(np.asarray(edge_index))

    if "P" not in _built:
        _built["P"] = _build_P()
    if ("L1", B) not in _built:
        _built[("L1", B)] = _build_conv(B, 512, 4, 256)
    if ("L2", B) not in _built:
        _built[("L2", B)] = _build_conv(B, 128, 1, 2)

    W1 = np.concatenate([Wk1, Wv1, Wq1, Ws1], axis=1).astype(np.float32)
    b1 = np.concatenate([bk1, bv1, bq1, bs1])[None, :].astype(np.float32)
    W2 = np.concatenate([Wk2, Wv2, Wq2, Ws2], axis=1).astype(np.float32)
    b2 = np.concatenate([bk2, bv2, bq2, bs2])[None, :].astype(np.float32)
    cids = list(range(NCORES))

    # ---- launch P: projections of own slices ----
    xTs = []
    for pc in per_core:
        xs = np.zeros((NOWN, 64), np.float32)
        xs[: pc["n1"] - pc["n0"]] = x[pc["n0"]: pc["n1"]]
        xTs.append(np.ascontiguousarray(xs.T))
    resP = run_bass_kernel_spmd(
        _built["P"],
        [{"xT": xTs[c], "W1": W1, "b1": b1} for c in cids], cids)
    tP = resP.exec_time_ns

    kv1 = np.concatenate(
        [resP.results[c]["kv"][: per_core[c]["n1"] - per_core[c]["n0"]]
         for c in cids], axis=0)                       # [N, 512]
    # ---- launch L1 ----
    in1 = []
    for c in cids:
        pc = per_core[c]
        qs = resP.results[c]["qs"]                     # [NOWN, 512] q|sk
        qtab = np.zeros((NOWN + 1, 256), np.float32)
        qtab[:NOWN] = qs[:, :256]
        in1.append(dict(kvf=kv1, qtab=qtab, sktab=np.ascontiguousarray(qs[:, 256:]),
                        srcb=pc["srcb"], Mb=pc["Mb"], qrow=pc["qrow"],
                        strow=pc["strow"], WO=W2, bO=b2))
    res1 = run_bass_kernel_spmd(_built[("L1", B)], in1, cids)
    t1 = res1.exec_time_ns

    kv2 = np.concatenate(
        [res1.results[c]["outt"][: per_core[c]["n1"] - per_core[c]["n0"], :128]
         for c in cids], axis=0)                       # [N, 128]
    # ---- launch L2 ----
    Wlc = np.asarray(Wl, np.float32)
    blc = np.asarray(bl, np.float32)[None, :]
    in2 = []
    for c in cids:
        pc = per_core[c]
        o1 = res1.results[c]["outt"]                   # [NOWN, 256] k2|v2|q2|sk2
        qtab2 = np.zeros((NOWN + 1, 64), np.float32)
        qtab2[:NOWN] = o1[:, 128:192]
        in2.append(dict(kvf=kv2, qtab=qtab2, sktab=np.ascontiguousarray(o1[:, 192:]),
                        srcb=pc["srcb"], Mb=pc["Mb"], qrow=pc["qrow"],
                        strow=pc["strow"], WO=Wlc, bO=blc))
    res2 = run_bass_kernel_spmd(_built[("L2", B)], in2, cids)
    t2 = res2.exec_time_ns

    out = np.concatenate(
        [res2.results[c]["outt"][: per_core[c]["n1"] - per_core[c]["n0"]]
         for c in cids], axis=0)
    kernel.exec_times = (tP, t1, t2)
    return out

